# revision 7
# baseline (speedup 1.0000x reference)
"""Trainium2 Bass kernel for nn_GaugeTheoryNode (8-core data-parallel MLP +
replicated Wilson-plaquette gather).

Reference computation (per full batch B=16384):
    h   = relu(x @ fw1 + fb1)          # [B, 2048]
    fo  = h @ fw2 + fb2                # [B, 4096]   (output 0)
    w   = relu(fo @ ww1 + wb1)         # [B, 2048]
    w   = relu(w @ ww2 + wb2)          # [B, 1024]
    wo  = w @ ww3 + wb3                # [B, 1]      (output 1)
    t   = relu(fo @ tw1 + tb1)         # [B, 2048]
    to  = t @ tw2 + tb2                # [B, 1]      (output 2)
    plaq = exp(sum of 24 signed entries of field gathered at x[0]-derived
               lattice coords)         # [1]         (output 3)

Sharding: batch split 8 ways (2048 rows/core), weights replicated, plaquette
computed replicated (kernel reads only core 0's copy back).

On-core layout: activations are feature-major ([feat, batch]) so each layer's
matmul takes the natural [K, M] weight slice as lhsT and the activation as
rhs, accumulating over K in PSUM. Weights stream HBM->SBUF in bf16.
field_output is written feature-major f32 and transposed on the host.
"""

import numpy as np
import ml_dtypes

import concourse.bass as bass
import concourse.tile as tile
from concourse import mybir, bacc
from concourse.bass_utils import run_bass_kernel_spmd

N_CORES = 8
B_FULL = 16384
B_CORE = B_FULL // N_CORES      # 2048
B_CHUNK = 512                   # pipeline chunk (SBUF residency)
N_CHUNKS = B_CORE // B_CHUNK    # 2
NT = B_CHUNK // 512             # 512-wide matmul n-tiles per chunk
P = 128

D_IN = 4
D_H1 = 2048     # h
D_FO = 4096     # field_output
D_W1 = 2048
D_W2 = 1024
L = 10          # lattice size

F32 = mybir.dt.float32
BF16 = mybir.dt.bfloat16
I32 = mybir.dt.int32
AF = mybir.ActivationFunctionType
OP = mybir.AluOpType

BF = ml_dtypes.bfloat16

# ---------------------------------------------------------------------------
# Plaquette constants.
#
# plaq = exp(total), total = sum over the 6 (mu<nu) pairs of
#   field[c, mu] + field[c+e_mu, nu] - field[c+e_mu+e_nu (spatial), mu]
#   - field[c, nu]
# with c = trunc(x[0, :3]) % 10 (4th coord never indexes the field).
# Expressed as a signed gather over 7 lattice sites x 4 components:
#   P0=c, P1=c+e0, P2=c+e1, P3=c+e2, P4=c+e0+e1, P5=c+e0+e2, P6=c+e1+e2
_PLAQ_COEF = np.array([
    [3.0, 1.0, -1.0, -3.0],    # P0
    [-1.0, 1.0, 1.0, 1.0],     # P1
    [0.0, -1.0, 1.0, 1.0],     # P2
    [0.0, 0.0, -1.0, 1.0],     # P3
    [-1.0, 0.0, 0.0, 0.0],     # P4
    [-1.0, 0.0, 0.0, 0.0],     # P5
    [0.0, -1.0, 0.0, 0.0],     # P6
], dtype=np.float32)

# Position j uses coord c_d (index d in cc8) or its +1-mod version (index
# 4+d), for d = 0 (a), 1 (b), 2 (c).
_POS_SHIFT = np.array([
    # (a from, b from, c from): 0 => c_d, 1 => c_d'
    [0, 0, 0],  # P0
    [1, 0, 0],  # P1
    [0, 1, 0],  # P2
    [0, 0, 1],  # P3
    [1, 1, 0],  # P4
    [1, 0, 1],  # P5
    [0, 1, 1],  # P6
], dtype=np.int64)


def _sel_matrix(dim: int) -> np.ndarray:
    """[7, 8] one-hot rows selecting position j's coord along lattice dim."""
    sel = np.zeros((7, 8), dtype=np.float32)
    for j in range(7):
        sel[j, dim + 4 * _POS_SHIFT[j, dim]] = 1.0
    return sel


def host_constants() -> dict[str, np.ndarray]:
    """Small replicated constant tensors fed as ordinary inputs."""
    tc19 = np.tile(np.arange(1, 10, dtype=np.float32), 4).reshape(1, 36)
    iota4 = np.tile(np.arange(4, dtype=np.float32), 7).reshape(1, 28)
    coef = np.zeros((P, 1), np.float32)
    coef[:28, 0] = _PLAQ_COEF.reshape(-1)
    ones = np.zeros((P, 1), np.float32)
    ones[:28, 0] = 1.0
    return {
        "tc19": tc19,
        "tc19n": -tc19,
        "selA": _sel_matrix(0).reshape(1, 56).copy(),
        "selB": _sel_matrix(1).reshape(1, 56).copy(),
        "selC": _sel_matrix(2).reshape(1, 56).copy(),
        "iota4c": iota4,
        "coefc": coef,
        "onesc": ones,
    }


# ---------------------------------------------------------------------------
# Program builder


def build_program():
    nc = bacc.Bacc("TRN2", target_bir_lowering=False, debug=False,
                   enable_asserts=True, num_devices=N_CORES)

    # --- I/O declarations -------------------------------------------------
    d_in = {}

    def din(name, shape, dtype):
        d_in[name] = nc.dram_tensor(name, list(shape), dtype,
                                    kind="ExternalInput")
        return d_in[name]

    xt_d = din("xt", [P, B_CORE], BF16)
    x0_d = din("x0", [1, D_IN], F32)
    field_d = din("field", [L * L * L * 4, 1], F32)

    fw1_d = din("fw1p", [P, D_H1], BF16)
    fw2_d = din("fw2", [D_H1, D_FO], BF16)
    ww1_d = din("ww1", [D_FO, D_W1], BF16)
    ww2_d = din("ww2", [D_W1, D_W2], BF16)
    ww3_d = din("ww3c", [P, D_W2 // P], BF16)
    tw1_d = din("tw1", [D_FO, D_H1], BF16)
    tw2_d = din("tw2c", [P, D_H1 // P], BF16)

    fb1_d = din("fb1c", [P, D_H1 // P], F32)
    fb2_d = din("fb2c", [P, D_FO // P], F32)
    wb1_d = din("wb1c", [P, D_W1 // P], F32)
    wb2_d = din("wb2c", [P, D_W2 // P], F32)
    wb3_d = din("wb3r", [1, 1], F32)
    tb1_d = din("tb1c", [P, D_H1 // P], F32)
    tb2_d = din("tb2r", [1, 1], F32)

    for name, arr in host_constants().items():
        din(name, arr.shape, F32)

    fo_out = nc.dram_tensor("fo_t", [D_FO, B_CORE], F32, kind="ExternalOutput")
    wo_out = nc.dram_tensor("wo", [B_CORE, 1], F32, kind="ExternalOutput")
    to_out = nc.dram_tensor("to", [B_CORE, 1], F32, kind="ExternalOutput")
    plaq_out = nc.dram_tensor("plaq", [1, 1], F32, kind="ExternalOutput")

    offs_scratch = nc.dram_tensor("offs_scratch", [28, 1], I32)  # Internal

    # Feature-major [P, K/P, M] views of the streamed weights.
    fw2_v = fw2_d.ap().rearrange("(ko p) m -> p ko m", p=P)
    ww1_v = ww1_d.ap().rearrange("(ko p) m -> p ko m", p=P)
    ww2_v = ww2_d.ap().rearrange("(ko p) m -> p ko m", p=P)
    tw1_v = tw1_d.ap().rearrange("(ko p) m -> p ko m", p=P)

    with tile.TileContext(nc) as tc:
        ctxs = []

        def pool(name, bufs, **kw):
            p = tc.tile_pool(name=name, bufs=bufs, **kw)
            ctxs.append(p)
            return p.__enter__()

        const_p = pool("const", 1)
        wpool = pool("w", 3)
        act_p = pool("act", 1)
        stg_p = pool("stg", 4)
        out1_p = pool("out1", 2)
        psum_p = pool("psum", 4, space="PSUM")
        psum1_p = pool("psum1", 2, space="PSUM")
        plq_p = pool("plq", 1)
        pps_p = pool("pps", 1, space="PSUM")

        # --- persistent SBUF loads ---------------------------------------
        def load_const(name, shape, dtype=F32):
            t = const_p.tile(list(shape), dtype, tag=name, name=name)
            nc.sync.dma_start(t[:], d_in[name].ap())
            return t

        xt = load_const("xt", [P, B_CORE], BF16)
        fw1p = load_const("fw1p", [P, D_H1], BF16)
        ww3c = load_const("ww3c", [P, D_W2 // P], BF16)
        tw2c = load_const("tw2c", [P, D_H1 // P], BF16)
        fb1c = load_const("fb1c", [P, D_H1 // P])
        fb2c = load_const("fb2c", [P, D_FO // P])
        wb1c = load_const("wb1c", [P, D_W1 // P])
        wb2c = load_const("wb2c", [P, D_W2 // P])
        wb3r = load_const("wb3r", [1, 1])
        tb1c = load_const("tb1c", [P, D_H1 // P])
        tb2r = load_const("tb2r", [1, 1])

        # ------------------------------------------------------------------
        # Wilson plaquette (tiny, overlaps the matmul pipeline)
        # ------------------------------------------------------------------
        x0_sb = load_const("x0", [1, D_IN])
        tc19 = load_const("tc19", [1, 36])
        tc19n = load_const("tc19n", [1, 36])
        selA = load_const("selA", [1, 56])
        selB = load_const("selB", [1, 56])
        selC = load_const("selC", [1, 56])
        iota4c = load_const("iota4c", [1, 28])
        coefc = load_const("coefc", [P, 1])
        onesc = load_const("onesc", [P, 1])

        def plq(shape, dtype=F32, tag=None):
            return plq_p.tile(list(shape), dtype, tag=tag, name=tag)

        # trunc(x) for |x| < 10, exact: #(x >= i) - #(x <= -i), i = 1..9
        ge = plq([1, 4, 9], tag="ge")
        x_b = x0_sb[:, :, None].to_broadcast([1, 4, 9])
        nc.vector.tensor_tensor(ge[:], x_b, tc19.rearrange("p (a b) -> p a b", b=9), OP.is_ge)
        sge = plq([1, 4], tag="sge")
        nc.vector.tensor_reduce(sge[:], ge[:], axis=mybir.AxisListType.X, op=OP.add)
        le = plq([1, 4, 9], tag="le")
        nc.vector.tensor_tensor(le[:], x_b, tc19n.rearrange("p (a b) -> p a b", b=9), OP.is_le)
        sle = plq([1, 4], tag="sle")
        nc.vector.tensor_reduce(sle[:], le[:], axis=mybir.AxisListType.X, op=OP.add)
        t_tr = plq([1, 4], tag="t_tr")
        nc.vector.tensor_tensor(t_tr[:], sge[:], sle[:], OP.subtract)

        # c = t + 10*(t < 0)   (python mod for |t| < 10)
        lt0 = plq([1, 4], tag="lt0")
        nc.vector.tensor_scalar(lt0[:], t_tr[:], 0.0, None, OP.is_lt)
        nc.vector.tensor_scalar(lt0[:], lt0[:], 10.0, None, OP.mult)
        cco = plq([1, 4], tag="cco")
        nc.vector.tensor_tensor(cco[:], t_tr[:], lt0[:], OP.add)

        # c' = (c+1) - 10*((c+1) >= 10)
        c1p = plq([1, 4], tag="c1p")
        nc.vector.tensor_scalar(c1p[:], cco[:], 1.0, None, OP.add)
        ge10 = plq([1, 4], tag="ge10")
        nc.vector.tensor_scalar(ge10[:], c1p[:], 10.0, None, OP.is_ge)
        nc.vector.tensor_scalar(ge10[:], ge10[:], 10.0, None, OP.mult)
        ccp = plq([1, 4], tag="ccp")
        nc.vector.tensor_tensor(ccp[:], c1p[:], ge10[:], OP.subtract)

        cc8 = plq([1, 8], tag="cc8")
        nc.vector.tensor_copy(out=cc8[:, 0:4], in_=cco[:])
        nc.vector.tensor_copy(out=cc8[:, 4:8], in_=ccp[:])

        # flat coords per position via one-hot selections
        cc_b = cc8[:, None, :].to_broadcast([1, 7, 8])

        def sel_reduce(sel, tag):
            tmp = plq([1, 7, 8], tag="seltmp")
            nc.vector.tensor_tensor(tmp[:], cc_b, sel.rearrange("p (j e) -> p j e", e=8), OP.mult)
            red = plq([1, 7], tag=tag)
            nc.vector.tensor_reduce(red[:], tmp[:], axis=mybir.AxisListType.X, op=OP.add)
            return red

        fa = sel_reduce(selA, "fa")
        fb = sel_reduce(selB, "fb")
        fc = sel_reduce(selC, "fc")
        # flat = ((a*10 + b)*10 + c) * 4
        nc.vector.tensor_scalar(fa[:], fa[:], 10.0, None, OP.mult)
        nc.vector.tensor_tensor(fa[:], fa[:], fb[:], OP.add)
        nc.vector.tensor_scalar(fa[:], fa[:], 10.0, None, OP.mult)
        nc.vector.tensor_tensor(fa[:], fa[:], fc[:], OP.add)
        nc.vector.tensor_scalar(fa[:], fa[:], 4.0, None, OP.mult)

        off28 = plq([1, 7, 4], tag="off28")
        nc.vector.tensor_tensor(off28[:], fa[:, :, None].to_broadcast([1, 7, 4]),
                                iota4c.rearrange("p (j e) -> p j e", e=4), OP.add)
        offi = plq([1, 28], I32, tag="offi")
        nc.vector.tensor_copy(out=offi[:], in_=off28.rearrange("p a b -> p (a b)"))

        # bounce through DRAM to move the 28 offsets onto 28 partitions
        nc.sync.dma_start(offs_scratch.ap().rearrange("f o -> o f"), offi[:])
        offp = plq([P, 1], I32, tag="offp")
        nc.sync.dma_start(offp[:28], offs_scratch.ap())

        gat = plq([P, 1], F32, tag="gat")
        nc.vector.memset(gat[:], 0.0)
        nc.gpsimd.indirect_dma_start(
            out=gat[:28],
            out_offset=None,
            in_=field_d.ap(),
            in_offset=bass.IndirectOffsetOnAxis(ap=offp[:28], axis=0),
        )
        prod = plq([P, 1], F32, tag="prod")
        nc.vector.tensor_tensor(prod[:], gat[:], coefc[:], OP.mult)
        pps = pps_p.tile([1, 1], F32, tag="pps")
        nc.tensor.matmul(pps[:], prod[:], onesc[:], start=True, stop=True)
        plaq_sb = plq([1, 1], F32, tag="plaq_sb")
        nc.scalar.activation(plaq_sb[:], pps[:], AF.Exp)
        nc.sync.dma_start(plaq_out.ap(), plaq_sb[:])

        # ------------------------------------------------------------------
        # Main MLP pipeline, per batch chunk
        # ------------------------------------------------------------------
        def dense(w_view, ksub, m_tiles, rhs, rhs_ks, out_cb, wtag):
            """out[m*128:(m+1)*128, n*512:(n+1)*512] = W[:, mslice].T @ rhs.

            w_view: [P, ksub, M] DRAM view; rhs: [P, rhs_ks, B_CHUNK] SBUF
            (only first `ksub` k-slices used -- rhs_ks == ksub).
            """
            assert rhs_ks == ksub
            for m in range(m_tiles):
                wt = wpool.tile([P, ksub, P], BF16, tag=wtag)
                nc.sync.dma_start(wt[:], w_view[:, :, m * P:(m + 1) * P])
                for n in range(NT):
                    ps = psum_p.tile([P, 512], F32, tag="ps")
                    for k in range(ksub):
                        nc.tensor.matmul(ps[:], wt[:, k, :],
                                         rhs[:, k, n * 512:(n + 1) * 512],
                                         start=(k == 0), stop=(k == ksub - 1))
                    out_cb(m, n, ps)

        for c in range(N_CHUNKS):
            cb = c * B_CHUNK

            # L1: ht = relu(fw1.T x + fb1) -> [P, 16, B_CHUNK] bf16
            ht = act_p.tile([P, D_H1 // P, B_CHUNK], BF16, tag="ht")
            for m in range(D_H1 // P):
                for n in range(NT):
                    ps = psum_p.tile([P, 512], F32, tag="ps")
                    nc.tensor.matmul(ps[:], fw1p[:, m * P:(m + 1) * P],
                                     xt[:, cb + n * 512:cb + (n + 1) * 512],
                                     start=True, stop=True)
                    nc.scalar.activation(ht[:, m, n * 512:(n + 1) * 512], ps[:],
                                         AF.Relu, bias=fb1c[:, m:m + 1])

            # L2: fo = ht.T-chain @ fw2 + fb2 (no relu)
            fo = act_p.tile([P, D_FO // P, B_CHUNK], BF16, tag="fo")

            def l2_out(m, n, ps):
                stage = stg_p.tile([P, 512], F32, tag="fostg")
                nc.vector.tensor_scalar(stage[:], ps[:], fb2c[:, m:m + 1], None, OP.add)
                nc.sync.dma_start(
                    fo_out.ap()[m * P:(m + 1) * P, cb + n * 512:cb + (n + 1) * 512],
                    stage[:])
                nc.scalar.activation(fo[:, m, n * 512:(n + 1) * 512], ps[:],
                                     AF.Identity, bias=fb2c[:, m:m + 1])

            dense(fw2_v, D_H1 // P, D_FO // P, ht, D_H1 // P, l2_out, "w16")

            # L3: w1 = relu(fo @ ww1 + wb1)
            w1 = act_p.tile([P, D_W1 // P, B_CHUNK], BF16, tag="w1")

            def l3_out(m, n, ps):
                nc.scalar.activation(w1[:, m, n * 512:(n + 1) * 512], ps[:],
                                     AF.Relu, bias=wb1c[:, m:m + 1])

            dense(ww1_v, D_FO // P, D_W1 // P, fo, D_FO // P, l3_out, "w32")

            # L4: w2 = relu(w1 @ ww2 + wb2)
            w2 = act_p.tile([P, D_W2 // P, B_CHUNK], BF16, tag="w2")

            def l4_out(m, n, ps):
                nc.scalar.activation(w2[:, m, n * 512:(n + 1) * 512], ps[:],
                                     AF.Relu, bias=wb2c[:, m:m + 1])

            dense(ww2_v, D_W1 // P, D_W2 // P, w1, D_W1 // P, l4_out, "w16")

            # L5: wo = w2 @ ww3 + wb3  -> [1, B_CHUNK]
            wo_stg = out1_p.tile([1, B_CHUNK], F32, tag="wostg")
            for n in range(NT):
                ps1 = psum1_p.tile([1, 512], F32, tag="ps1")
                for k in range(D_W2 // P):
                    nc.tensor.matmul(ps1[:], ww3c[:, k:k + 1],
                                     w2[:, k, n * 512:(n + 1) * 512],
                                     start=(k == 0), stop=(k == D_W2 // P - 1))
                nc.scalar.activation(wo_stg[:, n * 512:(n + 1) * 512], ps1[:],
                                     AF.Identity, bias=wb3r[:])
            nc.sync.dma_start(wo_out.ap()[cb:cb + B_CHUNK, :].rearrange("f o -> o f"),
                              wo_stg[:])

            # L6: t1 = relu(fo @ tw1 + tb1)
            t1 = act_p.tile([P, D_H1 // P, B_CHUNK], BF16, tag="t1")

            def l6_out(m, n, ps):
                nc.scalar.activation(t1[:, m, n * 512:(n + 1) * 512], ps[:],
                                     AF.Relu, bias=tb1c[:, m:m + 1])

            dense(tw1_v, D_FO // P, D_H1 // P, fo, D_FO // P, l6_out, "w32")

            # L7: to = t1 @ tw2 + tb2 -> [1, B_CHUNK]
            to_stg = out1_p.tile([1, B_CHUNK], F32, tag="tostg")
            for n in range(NT):
                ps1 = psum1_p.tile([1, 512], F32, tag="ps1")
                for k in range(D_H1 // P):
                    nc.tensor.matmul(ps1[:], tw2c[:, k:k + 1],
                                     t1[:, k, n * 512:(n + 1) * 512],
                                     start=(k == 0), stop=(k == D_H1 // P - 1))
                nc.scalar.activation(to_stg[:, n * 512:(n + 1) * 512], ps1[:],
                                     AF.Identity, bias=tb2r[:])
            nc.sync.dma_start(to_out.ap()[cb:cb + B_CHUNK, :].rearrange("f o -> o f"),
                              to_stg[:])

        for p_ in reversed(ctxs):
            p_.__exit__(None, None, None)

    nc.compile()
    return nc


_NC_CACHE = None


def _get_nc():
    global _NC_CACHE
    if _NC_CACHE is None:
        _NC_CACHE = build_program()
    return _NC_CACHE


# ---------------------------------------------------------------------------
# Host-side wrapper


def _prep_in_maps(x, field, fw1, fb1, fw2, fb2, ww1, wb1, ww2, wb2, ww3, wb3,
                  tw1, tb1, tw2, tb2):
    def col_fold(b):  # [K] -> [P, K/P] with column k = features k*P..k*P+P-1
        return np.ascontiguousarray(b.reshape(-1, P).T.astype(np.float32))

    shared = {
        "x0": np.ascontiguousarray(x[0, :4].reshape(1, 4).astype(np.float32)),
        "field": np.ascontiguousarray(field.reshape(-1, 1).astype(np.float32)),
        "fw2": np.ascontiguousarray(fw2.astype(BF)),
        "ww1": np.ascontiguousarray(ww1.astype(BF)),
        "ww2": np.ascontiguousarray(ww2.astype(BF)),
        "tw1": np.ascontiguousarray(tw1.astype(BF)),
        "ww3c": np.ascontiguousarray(ww3.reshape(-1, P).T.astype(BF)),
        "tw2c": np.ascontiguousarray(tw2.reshape(-1, P).T.astype(BF)),
        "fb1c": col_fold(fb1), "fb2c": col_fold(fb2),
        "wb1c": col_fold(wb1), "wb2c": col_fold(wb2),
        "tb1c": col_fold(tb1),
        "wb3r": np.asarray(wb3, np.float32).reshape(1, 1),
        "tb2r": np.asarray(tb2, np.float32).reshape(1, 1),
    }
    fw1p = np.zeros((P, D_H1), BF)
    fw1p[:4] = fw1.astype(BF)
    shared["fw1p"] = fw1p
    shared.update({k: v for k, v in host_constants().items()})

    in_maps = []
    for c in range(N_CORES):
        xs = x[c * B_CORE:(c + 1) * B_CORE, :4]
        xt = np.zeros((P, B_CORE), BF)
        xt[:4] = xs.T.astype(BF)
        m = dict(shared)
        m["xt"] = np.ascontiguousarray(xt)
        in_maps.append(m)
    return in_maps


def kernel(**inputs):
    nc = _get_nc()
    in_maps = _prep_in_maps(**{k: np.asarray(v) for k, v in inputs.items()})
    res = run_bass_kernel_spmd(nc, in_maps, core_ids=list(range(N_CORES)))

    fo_full = np.empty((B_FULL, D_FO), np.float32)
    wo_full = np.empty((B_FULL, 1), np.float32)
    to_full = np.empty((B_FULL, 1), np.float32)
    for c in range(N_CORES):
        r = res.results[c]
        fo_full[c * B_CORE:(c + 1) * B_CORE] = r["fo_t"].T
        wo_full[c * B_CORE:(c + 1) * B_CORE] = r["wo"]
        to_full[c * B_CORE:(c + 1) * B_CORE] = r["to"]
    plaq = res.results[0]["plaq"].reshape(1).astype(np.float32)
    return fo_full, wo_full, to_full, plaq
